# revision 24
# baseline (speedup 1.0000x reference)
"""Trainium2 Bass kernel for a 4-layer GPT-style transformer (B=2, S=1024,
D=512, H=8, DFF=2048, V=50257) sharded over 8 NeuronCores.  See kernel.py
docstring; this revision minimizes DMA count on the critical path."""

import numpy as np
import ml_dtypes

import concourse.bass as bass
import concourse.mybir as mybir
import concourse.tile as tile
from concourse import bacc
from concourse.bass_utils import run_bass_kernel_spmd
from concourse.masks import make_identity

AF = mybir.ActivationFunctionType
Alu = mybir.AluOpType
f32 = mybir.dt.float32
bf16 = mybir.dt.bfloat16

V, D, H, DK, DFF, L, B, S = 50257, 512, 8, 64, 2048, 4, 2, 1024
NC, P = 8, 128
EPS = 1e-5
TOK = 256
NT = TOK // P                # 2
KD = D // P                  # 4
KF = DFF // P                # 16
VS_PAD = 6400
VT_FULL = VS_PAD // P        # 50
VS = [6283] * 7 + [V - 7 * 6283]

_bf = lambda a: np.ascontiguousarray(np.asarray(a).astype(ml_dtypes.bfloat16))
_f32 = lambda a: np.ascontiguousarray(np.asarray(a, dtype=np.float32))


def _wload_ap(dram, kdim, n):
    """DRAM [kdim*P, n] viewed as dest-tile order [P, kdim, n]"""
    return bass.AP(tensor=dram, offset=0,
                   ap=[[n, P], [P * n, kdim], [1, n]])


def build(n_layers=L, vt=VT_FULL, debug=False, sim=False):
    nc = bacc.Bacc("TRN2", target_bir_lowering=False, debug=False, num_devices=NC)

    x0_in = nc.dram_tensor("x0", [TOK, D], f32, kind="ExternalInput")
    masks_in = nc.dram_tensor("masks", [4, P, P], bf16, kind="ExternalInput")
    Ws = []
    for l in range(n_layers):
        Ws.append({k: nc.dram_tensor(f"{k}{l}", shp, dt, kind="ExternalInput")
                   for k, shp, dt in [
                       ("wq", [D, D], bf16), ("wk", [D, D], bf16),
                       ("wv", [D, D], bf16), ("wo", [D, D], bf16),
                       ("w1", [D, DFF], bf16), ("w2", [DFF, D], bf16),
                       ("bvec", [P, 28], f32),      # bq|bk|bv|b1 per-partition
                       ("bod", [2, P, D], f32)]})   # bo_bcast | b2_bcast
    whead = nc.dram_tensor("whead", [D, vt * P], bf16, kind="ExternalInput")
    bhead = nc.dram_tensor("bhead", [vt * P], f32, kind="ExternalInput")
    logits_out = nc.dram_tensor("logitsT", [vt * P, B * S], f32, kind="ExternalOutput")
    xdbg = (nc.dram_tensor("xdbg", [n_layers, TOK, D], f32, kind="ExternalOutput")
            if debug else None)

    GROUPS_BATCH = [[0, 1, 2, 3], [4, 5, 6, 7]]
    GROUPS_ALL = [list(range(NC))]
    KVSZ = D * TOK + TOK * D     # KT_own + V_own elems per rank
    ags = []
    for l in range(n_layers):
        ai = nc.dram_tensor(f"agin{l}", [KVSZ], bf16)
        ao = nc.dram_tensor(f"agout{l}", [4 * KVSZ], bf16)
        ags.append((ai, ao))
    af_i = nc.dram_tensor("aginF", [D, TOK], bf16)
    af_o = nc.dram_tensor("agoutF", [NC * D, TOK], bf16, addr_space="Shared")

    import contextlib
    with tile.TileContext(nc) as tc, contextlib.ExitStack() as ctx:
        const = ctx.enter_context(tc.tile_pool(name="const", bufs=1))
        xp = ctx.enter_context(tc.tile_pool(name="xp", bufs=2))
        wp = ctx.enter_context(tc.tile_pool(name="wp", bufs=1))
        whp = ctx.enter_context(tc.tile_pool(name="whp", bufs=1))
        act = ctx.enter_context(tc.tile_pool(name="act", bufs=2))
        atn = ctx.enter_context(tc.tile_pool(name="atn", bufs=2))
        sm = ctx.enter_context(tc.tile_pool(name="sm", bufs=2))
        ps = ctx.enter_context(tc.tile_pool(name="ps", bufs=2, space="PSUM"))
        psu = ctx.enter_context(tc.tile_pool(name="psu", bufs=2, space="PSUM"))
        pss = ctx.enter_context(tc.tile_pool(name="pss", bufs=2, space="PSUM"))

        ident = const.tile([P, P], bf16)
        make_identity(nc, ident)
        ones_col = const.tile([P, 1], bf16)
        nc.vector.memset(ones_col, 1.0)
        eps_t = const.tile([P, 1], f32)
        nc.vector.memset(eps_t, EPS)
        masks = const.tile([P, 4, P], bf16)
        x_t = [xp.tile([P, D], f32, tag=f"x{t}", name=f"x_{t}") for t in range(NT)]
        with tc.high_priority():
            for t in range(NT):
                nc.sync.dma_start(out=x_t[t], in_=x0_in[t * P:(t + 1) * P, :])
            nc.sync.dma_start(out=masks, in_=bass.AP(
                tensor=masks_in, offset=0, ap=[[P, P], [P * P, 4], [1, P]]))

        def layernorm(src_tiles, tag):
            out = []
            for t in range(NT):
                stats = sm.tile([P, 6], f32, tag="stats")
                nc.vector.bn_stats(stats, src_tiles[t])
                mv = sm.tile([P, 2], f32, tag="mv")
                nc.vector.bn_aggr(mv, stats)
                sd = sm.tile([P, 1], f32, tag="sd")
                nc.scalar.activation(sd, mv[:, 1:2], AF.Sqrt, bias=eps_t, scale=1.0)
                nc.vector.reciprocal(sd, sd)
                h = act.tile([P, D], bf16, tag=f"{tag}{t}")
                nc.vector.tensor_scalar(
                    out=h, in0=src_tiles[t], scalar1=mv[:, 0:1], scalar2=sd,
                    op0=Alu.subtract, op1=Alu.mult)
                out.append(h)
            return out

        def transpose_own(h_tiles, tag):
            hT = act.tile([P, KD, TOK], bf16, tag=tag)
            for d in range(KD):
                for t in range(NT):
                    pt = ps.tile([P, P], bf16, tag="mm", bufs=2)
                    nc.tensor.transpose(pt, h_tiles[t][:, d * P:(d + 1) * P], ident)
                    nc.scalar.copy(hT[:, d, t * P:(t + 1) * P], pt)
            return hT

        def load_w(dram, kdim, ndim, tag, offset=700):
            off = min(offset, max(tc.cur_priority - 8, 0))
            with tc.high_priority(offset=off):
                wt = wp.tile([P, kdim, ndim], bf16, tag=tag, name=tag)
                nc.sync.dma_start(out=wt, in_=_wload_ap(dram, kdim, ndim))
            return wt

        for l in range(n_layers):
            W = Ws[l]
            ai, ao = ags[l]

            # ---- LN1, transpose ----
            h1 = layernorm(x_t, "h1_")
            hT_own = transpose_own(h1, "hTown")

            wq_sb = load_w(W["wq"], KD, D, "wq")
            wk_sb = load_w(W["wk"], KD, D, "wk")
            wv_sb = load_w(W["wv"], KD, D, "wv")
            bv_sb = sm.tile([P, 28], f32, tag="bvec")
            nc.sync.dma_start(out=bv_sb, in_=W["bvec"][:, :])
            bq_t = bv_sb[:, 0:4]; bk_t = bv_sb[:, 4:8]
            bvv_t = bv_sb[:, 8:12]; b1_t = bv_sb[:, 12:28]

            # ---- Q, K, V for OWN tokens only ----
            qT = atn.tile([P, KD, TOK], bf16, tag="qT")
            kT_own = atn.tile([P, KD, TOK], bf16, tag="kTown")
            for m in range(KD):
                pq = ps.tile([P, TOK], f32, tag="mm")
                for k in range(KD):
                    nc.tensor.matmul(pq, wq_sb[:, k, m * P:(m + 1) * P],
                                     hT_own[:, k, :],
                                     start=(k == 0), stop=(k == KD - 1))
                nc.scalar.activation(qT[:, m, :], pq, AF.Identity,
                                     bias=bq_t[:, m:m + 1], scale=1.0)
                pk = ps.tile([P, TOK], f32, tag="mm")
                for k in range(KD):
                    nc.tensor.matmul(pk, wk_sb[:, k, m * P:(m + 1) * P],
                                     hT_own[:, k, :],
                                     start=(k == 0), stop=(k == KD - 1))
                if m % 2 == 0:
                    nc.scalar.activation(kT_own[:, m, :], pk, AF.Identity,
                                         bias=bk_t[:, m:m + 1], scale=1.0)
                else:
                    nc.vector.tensor_scalar_add(out=kT_own[:, m, :], in0=pk,
                                                scalar1=bk_t[:, m:m + 1])
            v_own = atn.tile([P, NT, D], bf16, tag="vown")
            for t in range(NT):
                pv = ps.tile([P, D], f32, tag="mm")
                for k in range(KD):
                    nc.tensor.matmul(pv, hT_own[:, k, t * P:(t + 1) * P],
                                     wv_sb[:, k, :], start=(k == 0),
                                     stop=(k == KD - 1))
                if t % 2 == 0:
                    nc.vector.tensor_copy(v_own[:, t, :], pv)
                else:
                    nc.scalar.copy(v_own[:, t, :], pv)

            # ---- AllGather K,V within the batch group ----
            nc.sync.dma_start(
                out=bass.AP(tensor=ai, offset=0,
                            ap=[[TOK, P], [P * TOK, KD], [1, TOK]]),
                in_=kT_own)
            nc.sync.dma_start(
                out=bass.AP(tensor=ai, offset=D * TOK,
                            ap=[[D, P], [P * D, NT], [1, D]]),
                in_=v_own)
            if sim:
                for jp in range(4):
                    nc.sync.dma_start(out=ao[jp * KVSZ:(jp + 1) * KVSZ],
                                      in_=ai[:])
            else:
                nc.gpsimd.collective_compute(
                    "AllGather", Alu.bypass, replica_groups=GROUPS_BATCH,
                    ins=[ai.ap().opt()], outs=[ao.ap().opt()])

            kT = atn.tile([P, KD, 4 * TOK], bf16, tag="kT", bufs=1)
            v_all = atn.tile([P, 8, D], bf16, tag="vall", bufs=1)
            for jp in range(4):
                nc.sync.dma_start(
                    out=kT[:, :, jp * TOK:(jp + 1) * TOK],
                    in_=bass.AP(tensor=ao, offset=jp * KVSZ,
                                ap=[[TOK, P], [P * TOK, KD], [1, TOK]]))
                nc.sync.dma_start(
                    out=v_all[:, 2 * jp:2 * jp + 2, :],
                    in_=bass.AP(tensor=ao, offset=jp * KVSZ + D * TOK,
                                ap=[[D, P], [P * D, NT], [1, D]]))

            # ---- attention per head ----
            oT = atn.tile([P, KD, TOK], bf16, tag="oT")
            for h in range(H):
                mt, bp = h // 2, 64 * (h % 2)
                kh = lambda col0, n: kT[bp:bp + DK, mt, col0:col0 + n]
                qh = qT[bp:bp + DK, mt, :]

                pT0 = atn.tile([P, 4, TOK], bf16, tag="pT0")
                pT1 = atn.tile([P, 4, P], bf16, tag="pT1")
                for pr in range(2):
                    sc = pss.tile([P, 2 * TOK], f32, tag="sc", bufs=3)
                    for i in range(2):
                        jp = 2 * pr + i
                        nc.tensor.matmul(sc[:, i * TOK:(i + 1) * TOK],
                                         kh(256 * jp, P), qh,
                                         start=True, stop=True,
                                         skip_group_check=True)
                    nc.scalar.activation(pT0[:, 2 * pr:2 * pr + 2, :], sc, AF.Exp)
                    for i in range(2):
                        jp = 2 * pr + i
                        nc.vector.tensor_mul(pT0[:, jp, 0:P], pT0[:, jp, 0:P],
                                             masks[:, jp, :])
                for pr in range(2):
                    sc1 = pss.tile([P, TOK], f32, tag="sc", bufs=3)
                    for i in range(2):
                        jp = 2 * pr + i
                        nc.tensor.matmul(sc1[:, i * P:(i + 1) * P],
                                         kh(256 * jp + P, P), qh[:, P:TOK],
                                         start=True, stop=True,
                                         skip_group_check=True)
                    nc.scalar.activation(pT1[:, 2 * pr:2 * pr + 2, :], sc1, AF.Exp)
                    for i in range(2):
                        jp = 2 * pr + i
                        nc.vector.tensor_mul(pT1[:, jp, :], pT1[:, jp, :],
                                             masks[:, jp, :])

                # denominators
                pd = pss.tile([1, TOK], f32, tag="pd", bufs=1)
                for jp in range(4):
                    nc.tensor.matmul(pd, ones_col, pT0[:, jp, :],
                                     start=(jp == 0), stop=False,
                                     skip_group_check=True)
                for jp in range(4):
                    nc.tensor.matmul(pd[:, P:TOK], ones_col, pT1[:, jp, :],
                                     start=False, stop=(jp == 3),
                                     skip_group_check=True)
                sums = sm.tile([1, TOK], f32, tag="sums", bufs=2)
                nc.scalar.copy(sums, pd)

                # u^T accumulation
                pu = psu.tile([DK, TOK], f32, tag="pu")
                vh = lambda i: v_all[:, i, h * DK:(h + 1) * DK]
                for jp in range(4):
                    nc.tensor.matmul(pu, vh(2 * jp), pT0[:, jp, :],
                                     start=(jp == 0), stop=False,
                                     skip_group_check=True)
                for jp in range(4):
                    nc.tensor.matmul(pu[:, P:TOK], vh(2 * jp + 1), pT1[:, jp, :],
                                     start=False, stop=(jp == 3),
                                     skip_group_check=True)

                rec = sm.tile([1, TOK], f32, tag="rec", bufs=2)
                nc.vector.reciprocal(rec, sums)
                recb = sm.tile([DK, TOK], f32, tag="recb", bufs=2)
                nc.gpsimd.partition_broadcast(recb, rec)
                nc.vector.tensor_mul(oT[bp:bp + DK, mt, :], pu, recb)
                nc.vector.tensor_scalar_add(
                    out=oT[bp:bp + DK, mt, :], in0=oT[bp:bp + DK, mt, :],
                    scalar1=bvv_t[bp:bp + DK, mt:mt + 1])

            # ---- attention out-projection + residual ----
            wo_sb = load_w(W["wo"], KD, D, "wo")
            bod_sb = wp.tile([P, 2, D], f32, tag="bod")
            nc.sync.dma_start(out=bod_sb, in_=bass.AP(
                tensor=W["bod"], offset=0, ap=[[D, P], [P * D, 2], [1, D]]))
            xb_t = []
            for t in range(NT):
                xb = xp.tile([P, D], f32, tag=f"xb{t}", bufs=1)
                nc.vector.tensor_add(xb, x_t[t], bod_sb[:, 0, :])
                xb_t.append(xb)
            for t in range(NT):
                py = ps.tile([P, D], f32, tag="mm")
                for k in range(KD):
                    nc.tensor.matmul(py, oT[:, k, t * P:(t + 1) * P],
                                     wo_sb[:, k, :],
                                     start=(k == 0), stop=(k == KD - 1))
                xn = xp.tile([P, D], f32, tag=f"x{t}")
                nc.vector.tensor_add(xn, py, xb_t[t])
                x_t[t] = xn

            # ---- FFN ----
            h2 = layernorm(x_t, "h2_")
            h2T = transpose_own(h2, "h2T")
            w1_sb = load_w(W["w1"], KD, DFF, "w1", offset=400)
            w2_sb = load_w(W["w2"], KF, D, "w2", offset=400)
            gT = act.tile([P, KF, TOK], bf16, tag="gT", bufs=1)
            xb2_t = []
            for t in range(NT):
                xb = xp.tile([P, D], f32, tag=f"xc{t}", bufs=1)
                nc.vector.tensor_add(xb, x_t[t], bod_sb[:, 1, :])
                xb2_t.append(xb)
            for m in range(KF):
                pa = ps.tile([P, TOK], f32, tag="mm")
                for k in range(KD):
                    nc.tensor.matmul(pa, w1_sb[:, k, m * P:(m + 1) * P],
                                     h2T[:, k, :],
                                     start=(k == 0), stop=(k == KD - 1))
                nc.scalar.activation(gT[:, m, :], pa, AF.Gelu,
                                     bias=b1_t[:, m:m + 1], scale=1.0)
            for t in range(NT):
                pz = ps.tile([P, D], f32, tag="mm")
                for k in range(KF):
                    nc.tensor.matmul(pz, gT[:, k, t * P:(t + 1) * P],
                                     w2_sb[:, k, :],
                                     start=(k == 0), stop=(k == KF - 1))
                xn = xp.tile([P, D], f32, tag=f"x{t}")
                nc.vector.tensor_add(xn, pz, xb2_t[t])
                x_t[t] = xn
                if debug:
                    nc.sync.dma_start(out=xdbg[l, t * P:(t + 1) * P, :], in_=xn)

        # ---- final LN + 8-way AllGather + lm_head ----
        hf = layernorm(x_t, "hf_")
        hfT = transpose_own(hf, "hfT")
        nc.sync.dma_start(
            out=bass.AP(tensor=af_i, offset=0,
                        ap=[[TOK, P], [P * TOK, KD], [1, TOK]]),
            in_=hfT)
        if sim:
            for rk in range(NC):
                nc.sync.dma_start(out=af_o[rk * D:(rk + 1) * D, :], in_=af_i[:, :])
        else:
            nc.gpsimd.collective_compute(
                "AllGather", Alu.bypass, replica_groups=GROUPS_ALL,
                ins=[af_i.ap().opt()], outs=[af_o.ap().opt()])

        xfT = act.tile([P, KD, NC * TOK], bf16, tag="xfT", bufs=1)
        for rk in range(NC):
            nc.sync.dma_start(
                out=xfT[:, :, rk * TOK:(rk + 1) * TOK],
                in_=bass.AP(tensor=af_o, offset=rk * D * TOK,
                            ap=[[TOK, P], [P * TOK, KD], [1, TOK]]))

        bh_t = sm.tile([P, vt], f32, tag="bh")
        nc.sync.dma_start(out=bh_t, in_=bass.AP(
            tensor=bhead, offset=0, ap=[[1, P], [P, vt]]))
        NCHUNK = (B * S) // 512
        MC = 10
        for m0 in range(0, vt, MC):
            mn = min(MC, vt - m0)
            woff = min(1500, max(tc.cur_priority - 8, 0))
            with tc.high_priority(offset=woff):
                whc = whp.tile([P, KD, MC * P], bf16, tag="wh", bufs=2)
                nc.sync.dma_start(
                    out=whc[:, :, :mn * P],
                    in_=bass.AP(tensor=whead, offset=m0 * P,
                                ap=[[vt * P, P], [P * vt * P, KD], [1, mn * P]]))
            for mi in range(mn):
                m = m0 + mi
                lo = act.tile([P, B * S], f32, tag=f"lo{m % 2}", bufs=1)
                for c2 in range(NCHUNK):
                    if (m + c2) % 2 == 0:
                        pl = ps.tile([P, 512], f32, tag="mm")
                    else:
                        pl = pss.tile([P, 512], f32, tag="sc", bufs=3)
                    for k in range(KD):
                        nc.tensor.matmul(
                            pl, whc[:, k, mi * P:(mi + 1) * P],
                            xfT[:, k, c2 * 512:(c2 + 1) * 512],
                            start=(k == 0), stop=(k == KD - 1))
                    dst = lo[:, c2 * 512:(c2 + 1) * 512]
                    if (m + c2) % 2 == 0:
                        nc.scalar.activation(dst, pl, AF.Identity,
                                             bias=bh_t[:, m:m + 1], scale=1.0)
                    else:
                        nc.vector.tensor_scalar_add(out=dst, in0=pl,
                                                    scalar1=bh_t[:, m:m + 1])
                nc.sync.dma_start(out=logits_out[m * P:(m + 1) * P, :], in_=lo)

    nc.compile()
    return nc


# --------------------------------------------------------------------------
# host side
# --------------------------------------------------------------------------

def host_prep(inputs, n_layers=L, vt=VT_FULL):
    emb = _f32(inputs["embedding"])
    pos = _f32(inputs["pos_embedding"])[0, :S]
    tokens = np.asarray(inputs["tokens"]).astype(np.int64)

    g1 = _f32(inputs["ln1_g"]); b1l = _f32(inputs["ln1_b"])
    g2 = _f32(inputs["ln2_g"]); b2l = _f32(inputs["ln2_b"])
    gf = _f32(inputs["lnf_g"]); bfl = _f32(inputs["lnf_b"])

    shared = {}
    for l in range(n_layers):
        Wq, Wk, Wv, Wo = (_f32(inputs[k][l]) for k in ["Wq", "Wk", "Wv", "Wo"])
        W1, W2 = _f32(inputs["W1"][l]), _f32(inputs["W2"][l])
        bq, bk, bv, bo = (_f32(inputs[k][l]) for k in ["bq", "bk", "bv", "bo"])
        b1, b2 = _f32(inputs["b1"][l]), _f32(inputs["b2"][l])
        shared[f"wq{l}"] = _bf(((Wq * g1[l]) / 8.0).T)
        shared[f"wk{l}"] = _bf((Wk * g1[l]).T)
        shared[f"wv{l}"] = _bf((Wv * g1[l]).T)
        shared[f"wo{l}"] = _bf(Wo.T)
        shared[f"w1{l}"] = _bf((W1 * g2[l]).T)
        shared[f"w2{l}"] = _bf(W2.T)
        bq_e = (bq + Wq @ b1l[l]) / 8.0
        bk_e = bk + Wk @ b1l[l]
        bv_e = bv + Wv @ b1l[l]
        b1_e = b1 + W1 @ b2l[l]
        bvec = np.zeros((P, 28), np.float32)
        bvec[:, 0:4] = bq_e.reshape(4, P).T
        bvec[:, 4:8] = bk_e.reshape(4, P).T
        bvec[:, 8:12] = bv_e.reshape(4, P).T
        bvec[:, 12:28] = b1_e.reshape(16, P).T
        shared[f"bvec{l}"] = bvec
        bod = np.zeros((2, P, D), np.float32)
        bod[0] = np.broadcast_to(bo, (P, D))
        bod[1] = np.broadcast_to(b2, (P, D))
        shared[f"bod{l}"] = bod

    Whead = _f32(inputs["Whead"]); bh = _f32(inputs["bhead"])
    Whead_eff = Whead * gf
    bh_eff = bh + Whead @ bfl

    in_maps = []
    for c in range(NC):
        b, j = c // 4, c % 4
        m = {"x0": np.zeros((TOK, D), np.float32)}
        for g in range(NT):
            t_ids = 512 * g + 4 * np.arange(P) + j
            m["x0"][g * P:(g + 1) * P] = emb[tokens[b, t_ids]] + pos[t_ids]
        mk = np.zeros((4, P, P), np.float32)
        for jp in range(4):
            rk = np.arange(P)[:, None]; rq = np.arange(P)[None, :]
            mk[jp] = (rk <= rq - (1 if jp > j else 0)).astype(np.float32)
        m["masks"] = _bf(mk)
        v0 = sum(VS[:c])
        n = min(VS[c], vt * P)
        wslice = np.zeros((D, vt * P), np.float32)
        bslice = np.zeros((vt * P,), np.float32)
        wslice[:, :n] = Whead_eff.T[:, v0:v0 + n]
        bslice[:n] = bh_eff[v0:v0 + n]
        m["whead"] = _bf(wslice)
        m["bhead"] = _f32(bslice)
        m.update(shared)
        in_maps.append(m)
    return in_maps


def assemble(results, vt=VT_FULL):
    gam = np.arange(NC * TOK)
    cp = gam // TOK; w = gam % TOK
    gp = w // P; rp = w % P
    bb = cp // 4; jj = cp % 4
    t = 512 * gp + 4 * rp + jj
    rows = bb * S + t
    out = np.empty((B * S, V), np.float32)
    for c in range(NC):
        v0 = sum(VS[:c])
        lt = results[c]["logitsT"][:VS[c]]
        out[rows, v0:v0 + VS[c]] = lt.T
    return out.reshape(B, S, V)


_CACHE = {}


def kernel(**inputs):
    key = ("full", L, VT_FULL)
    if key not in _CACHE:
        _CACHE[key] = build(L, VT_FULL, debug=False)
    nc = _CACHE[key]
    in_maps = host_prep(inputs, L, VT_FULL)
    res = run_bass_kernel_spmd(nc, in_maps, list(range(NC)))
    return assemble(res.results, VT_FULL)


# revision 27
# speedup vs baseline: 13276.0836x; 13276.0836x over previous
"""Trainium2 Bass kernel for a 4-layer GPT-style transformer (B=2, S=1024,
D=512, H=8, DFF=2048, V=50257) sharded over 8 NeuronCores.  See kernel.py
docstring; this revision minimizes DMA count on the critical path."""

import numpy as np
import ml_dtypes

import concourse.bass as bass
import concourse.mybir as mybir
import concourse.tile as tile
from concourse import bacc
from concourse.bass_utils import run_bass_kernel_spmd
from concourse.masks import make_identity

AF = mybir.ActivationFunctionType
Alu = mybir.AluOpType
f32 = mybir.dt.float32
bf16 = mybir.dt.bfloat16

V, D, H, DK, DFF, L, B, S = 50257, 512, 8, 64, 2048, 4, 2, 1024
NC, P = 8, 128
EPS = 1e-5
TOK = 256
NT = TOK // P                # 2
KD = D // P                  # 4
KF = DFF // P                # 16
VS_PAD = 6400
VT_FULL = VS_PAD // P        # 50
VS = [6283] * 7 + [V - 7 * 6283]

_bf = lambda a: np.ascontiguousarray(np.asarray(a).astype(ml_dtypes.bfloat16))
_f32 = lambda a: np.ascontiguousarray(np.asarray(a, dtype=np.float32))


def _wload_ap(dram, kdim, n):
    """DRAM [kdim*P, n] viewed as dest-tile order [P, kdim, n]"""
    return bass.AP(tensor=dram, offset=0,
                   ap=[[n, P], [P * n, kdim], [1, n]])


def build(n_layers=L, vt=VT_FULL, debug=False, sim=False):
    nc = bacc.Bacc("TRN2", target_bir_lowering=False, debug=False, num_devices=NC)

    x0_in = nc.dram_tensor("x0", [TOK, D], f32, kind="ExternalInput")
    masks_in = nc.dram_tensor("masks", [4, P, P], bf16, kind="ExternalInput")
    Ws = []
    for l in range(n_layers):
        Ws.append({k: nc.dram_tensor(f"{k}{l}", shp, dt, kind="ExternalInput")
                   for k, shp, dt in [
                       ("wq", [D, D], bf16), ("wk", [D, D], bf16),
                       ("wv", [D, D], bf16), ("wo", [D, D], bf16),
                       ("w1", [D, DFF], bf16), ("w2", [DFF, D], bf16),
                       ("bvec", [P, 28], f32),      # bq|bk|bv|b1 per-partition
                       ("bod", [2, P, D], f32)]})   # bo_bcast | b2_bcast
    whead = nc.dram_tensor("whead", [D, vt * P], bf16, kind="ExternalInput")
    bhead = nc.dram_tensor("bhead", [vt * P], f32, kind="ExternalInput")
    logits_out = nc.dram_tensor("logitsT", [vt * P, B * S], f32, kind="ExternalOutput")
    xdbg = (nc.dram_tensor("xdbg", [n_layers, TOK, D], f32, kind="ExternalOutput")
            if debug else None)

    GROUPS_BATCH = [[0, 1, 2, 3], [4, 5, 6, 7]]
    GROUPS_ALL = [list(range(NC))]
    KVSZ = D * TOK + TOK * D     # KT_own + V_own elems per rank
    ags = []
    for l in range(n_layers):
        ai = nc.dram_tensor(f"agin{l}", [KVSZ], bf16)
        ao = nc.dram_tensor(f"agout{l}", [4 * KVSZ], bf16)
        ags.append((ai, ao))
    af_i = nc.dram_tensor("aginF", [D, TOK], bf16)
    af_o = nc.dram_tensor("agoutF", [NC * D, TOK], bf16, addr_space="Shared")

    import contextlib
    with tile.TileContext(nc) as tc, contextlib.ExitStack() as ctx:
        const = ctx.enter_context(tc.tile_pool(name="const", bufs=1))
        xp = ctx.enter_context(tc.tile_pool(name="xp", bufs=2))
        wp = ctx.enter_context(tc.tile_pool(name="wp", bufs=1))
        whp = ctx.enter_context(tc.tile_pool(name="whp", bufs=1))
        act = ctx.enter_context(tc.tile_pool(name="act", bufs=2))
        atn = ctx.enter_context(tc.tile_pool(name="atn", bufs=2))
        sm = ctx.enter_context(tc.tile_pool(name="sm", bufs=2))
        ps = ctx.enter_context(tc.tile_pool(name="ps", bufs=3, space="PSUM"))
        psu = ctx.enter_context(tc.tile_pool(name="psu", bufs=2, space="PSUM"))
        pss = ctx.enter_context(tc.tile_pool(name="pss", bufs=2, space="PSUM"))

        ident = const.tile([P, P], bf16)
        make_identity(nc, ident)
        ones_col = const.tile([P, 1], bf16)
        nc.vector.memset(ones_col, 1.0)
        eps_t = const.tile([P, 1], f32)
        nc.vector.memset(eps_t, EPS)
        masks = const.tile([P, 4, P], bf16)
        x_t = [xp.tile([P, D], f32, tag=f"x{t}", name=f"x_{t}") for t in range(NT)]
        with tc.high_priority():
            for t in range(NT):
                nc.sync.dma_start(out=x_t[t], in_=x0_in[t * P:(t + 1) * P, :])
            nc.sync.dma_start(out=masks, in_=bass.AP(
                tensor=masks_in, offset=0, ap=[[P, P], [P * P, 4], [1, P]]))

        def layernorm(src_tiles, tag):
            out = []
            for t in range(NT):
                stats = sm.tile([P, 6], f32, tag="stats")
                nc.vector.bn_stats(stats, src_tiles[t])
                mv = sm.tile([P, 2], f32, tag="mv")
                nc.vector.bn_aggr(mv, stats)
                sd = sm.tile([P, 1], f32, tag="sd")
                nc.scalar.activation(sd, mv[:, 1:2], AF.Sqrt, bias=eps_t, scale=1.0)
                nc.vector.reciprocal(sd, sd)
                h = act.tile([P, D], bf16, tag=f"{tag}{t}")
                nc.vector.tensor_scalar(
                    out=h, in0=src_tiles[t], scalar1=mv[:, 0:1], scalar2=sd,
                    op0=Alu.subtract, op1=Alu.mult)
                out.append(h)
            return out

        def transpose_own(h_tiles, tag):
            hT = act.tile([P, KD, TOK], bf16, tag=tag)
            for d in range(KD):
                for t in range(NT):
                    pt = ps.tile([P, P], bf16, tag="mm", bufs=3)
                    nc.tensor.transpose(pt, h_tiles[t][:, d * P:(d + 1) * P], ident)
                    nc.scalar.copy(hT[:, d, t * P:(t + 1) * P], pt)
            return hT

        def load_w(dram, kdim, ndim, tag, offset=700):
            off = min(offset, max(tc.cur_priority - 8, 0))
            with tc.high_priority(offset=off):
                wt = wp.tile([P, kdim, ndim], bf16, tag=tag, name=tag)
                nc.sync.dma_start(out=wt, in_=_wload_ap(dram, kdim, ndim))
            return wt

        for l in range(n_layers):
            W = Ws[l]
            ai, ao = ags[l]

            # ---- LN1, transpose ----
            h1 = layernorm(x_t, "h1_")
            hT_own = transpose_own(h1, "hTown")

            wq_sb = load_w(W["wq"], KD, D, "wq")
            wk_sb = load_w(W["wk"], KD, D, "wk")
            wv_sb = load_w(W["wv"], KD, D, "wv")
            bv_sb = sm.tile([P, 28], f32, tag="bvec")
            nc.sync.dma_start(out=bv_sb, in_=W["bvec"][:, :])
            bq_t = bv_sb[:, 0:4]; bk_t = bv_sb[:, 4:8]
            bvv_t = bv_sb[:, 8:12]; b1_t = bv_sb[:, 12:28]

            # ---- Q, K, V for OWN tokens only ----
            qT = atn.tile([P, KD, TOK], bf16, tag="qT")
            kT_own = atn.tile([P, KD, TOK], bf16, tag="kTown")
            for m in range(KD):
                pq = ps.tile([P, TOK], f32, tag="mm")
                for k in range(KD):
                    nc.tensor.matmul(pq, wq_sb[:, k, m * P:(m + 1) * P],
                                     hT_own[:, k, :],
                                     start=(k == 0), stop=(k == KD - 1))
                nc.scalar.activation(qT[:, m, :], pq, AF.Identity,
                                     bias=bq_t[:, m:m + 1], scale=1.0)
                pk = ps.tile([P, TOK], f32, tag="mm")
                for k in range(KD):
                    nc.tensor.matmul(pk, wk_sb[:, k, m * P:(m + 1) * P],
                                     hT_own[:, k, :],
                                     start=(k == 0), stop=(k == KD - 1))
                if m % 2 == 0:
                    nc.scalar.activation(kT_own[:, m, :], pk, AF.Identity,
                                         bias=bk_t[:, m:m + 1], scale=1.0)
                else:
                    nc.vector.tensor_scalar_add(out=kT_own[:, m, :], in0=pk,
                                                scalar1=bk_t[:, m:m + 1])
            v_own = atn.tile([P, NT, D], bf16, tag="vown")
            for t in range(NT):
                pv = ps.tile([P, D], f32, tag="mm")
                for k in range(KD):
                    nc.tensor.matmul(pv, hT_own[:, k, t * P:(t + 1) * P],
                                     wv_sb[:, k, :], start=(k == 0),
                                     stop=(k == KD - 1))
                if t % 2 == 0:
                    nc.vector.tensor_copy(v_own[:, t, :], pv)
                else:
                    nc.scalar.copy(v_own[:, t, :], pv)

            # ---- AllGather K,V within the batch group ----
            nc.sync.dma_start(
                out=bass.AP(tensor=ai, offset=0,
                            ap=[[TOK, P], [P * TOK, KD], [1, TOK]]),
                in_=kT_own)
            nc.sync.dma_start(
                out=bass.AP(tensor=ai, offset=D * TOK,
                            ap=[[D, P], [P * D, NT], [1, D]]),
                in_=v_own)
            if sim:
                for jp in range(4):
                    nc.sync.dma_start(out=ao[jp * KVSZ:(jp + 1) * KVSZ],
                                      in_=ai[:])
            else:
                nc.gpsimd.collective_compute(
                    "AllGather", Alu.bypass, replica_groups=GROUPS_BATCH,
                    ins=[ai.ap().opt()], outs=[ao.ap().opt()])

            kT = atn.tile([P, KD, 4 * TOK], bf16, tag="kT", bufs=1)
            v_all = atn.tile([P, 8, D], bf16, tag="vall", bufs=1)
            for jp in range(4):
                nc.sync.dma_start(
                    out=kT[:, :, jp * TOK:(jp + 1) * TOK],
                    in_=bass.AP(tensor=ao, offset=jp * KVSZ,
                                ap=[[TOK, P], [P * TOK, KD], [1, TOK]]))
                nc.sync.dma_start(
                    out=v_all[:, 2 * jp:2 * jp + 2, :],
                    in_=bass.AP(tensor=ao, offset=jp * KVSZ + D * TOK,
                                ap=[[D, P], [P * D, NT], [1, D]]))

            # ---- attention per head ----
            oT = atn.tile([P, KD, TOK], bf16, tag="oT")
            for h in range(H):
                mt, bp = h // 2, 64 * (h % 2)
                kh = lambda col0, n: kT[bp:bp + DK, mt, col0:col0 + n]
                qh = qT[bp:bp + DK, mt, :]

                pT0 = atn.tile([P, 4, TOK], bf16, tag="pT0")
                pT1 = atn.tile([P, 4, P], bf16, tag="pT1")
                for pr in range(2):
                    sc = pss.tile([P, 2 * TOK], f32, tag="sc", bufs=2)
                    for i in range(2):
                        jp = 2 * pr + i
                        nc.tensor.matmul(sc[:, i * TOK:(i + 1) * TOK],
                                         kh(256 * jp, P), qh,
                                         start=True, stop=True,
                                         skip_group_check=True)
                    nc.scalar.activation(pT0[:, 2 * pr:2 * pr + 2, :], sc, AF.Exp)
                    for i in range(2):
                        jp = 2 * pr + i
                        nc.vector.tensor_mul(pT0[:, jp, 0:P], pT0[:, jp, 0:P],
                                             masks[:, jp, :])
                for pr in range(2):
                    sc1 = pss.tile([P, TOK], f32, tag="sc", bufs=2)
                    for i in range(2):
                        jp = 2 * pr + i
                        nc.tensor.matmul(sc1[:, i * P:(i + 1) * P],
                                         kh(256 * jp + P, P), qh[:, P:TOK],
                                         start=True, stop=True,
                                         skip_group_check=True)
                    nc.scalar.activation(pT1[:, 2 * pr:2 * pr + 2, :], sc1, AF.Exp)
                    for i in range(2):
                        jp = 2 * pr + i
                        nc.vector.tensor_mul(pT1[:, jp, :], pT1[:, jp, :],
                                             masks[:, jp, :])

                # denominators
                pd = pss.tile([1, TOK], f32, tag="pd", bufs=1)
                for jp in range(4):
                    nc.tensor.matmul(pd, ones_col, pT0[:, jp, :],
                                     start=(jp == 0), stop=False,
                                     skip_group_check=True)
                for jp in range(4):
                    nc.tensor.matmul(pd[:, P:TOK], ones_col, pT1[:, jp, :],
                                     start=False, stop=(jp == 3),
                                     skip_group_check=True)
                sums = sm.tile([1, TOK], f32, tag="sums", bufs=2)
                nc.scalar.copy(sums, pd)

                # u^T accumulation
                pu = psu.tile([DK, TOK], f32, tag="pu")
                vh = lambda i: v_all[:, i, h * DK:(h + 1) * DK]
                for jp in range(4):
                    nc.tensor.matmul(pu, vh(2 * jp), pT0[:, jp, :],
                                     start=(jp == 0), stop=False,
                                     skip_group_check=True)
                for jp in range(4):
                    nc.tensor.matmul(pu[:, P:TOK], vh(2 * jp + 1), pT1[:, jp, :],
                                     start=False, stop=(jp == 3),
                                     skip_group_check=True)

                rec = sm.tile([1, TOK], f32, tag="rec", bufs=2)
                nc.vector.reciprocal(rec, sums)
                recb = sm.tile([DK, TOK], f32, tag="recb", bufs=2)
                nc.gpsimd.partition_broadcast(recb, rec)
                nc.vector.tensor_mul(oT[bp:bp + DK, mt, :], pu, recb)
                nc.vector.tensor_scalar_add(
                    out=oT[bp:bp + DK, mt, :], in0=oT[bp:bp + DK, mt, :],
                    scalar1=bvv_t[bp:bp + DK, mt:mt + 1])

            # ---- attention out-projection + residual ----
            wo_sb = load_w(W["wo"], KD, D, "wo")
            bod_sb = wp.tile([P, 2, D], f32, tag="bod")
            nc.sync.dma_start(out=bod_sb, in_=bass.AP(
                tensor=W["bod"], offset=0, ap=[[D, P], [P * D, 2], [1, D]]))
            xb_t = []
            for t in range(NT):
                xb = xp.tile([P, D], f32, tag=f"xb{t}", bufs=1)
                nc.vector.tensor_add(xb, x_t[t], bod_sb[:, 0, :])
                xb_t.append(xb)
            for t in range(NT):
                py = ps.tile([P, D], f32, tag="mm")
                for k in range(KD):
                    nc.tensor.matmul(py, oT[:, k, t * P:(t + 1) * P],
                                     wo_sb[:, k, :],
                                     start=(k == 0), stop=(k == KD - 1))
                xn = xp.tile([P, D], f32, tag=f"x{t}")
                nc.vector.tensor_add(xn, py, xb_t[t])
                x_t[t] = xn

            # ---- FFN ----
            h2 = layernorm(x_t, "h2_")
            h2T = transpose_own(h2, "h2T")
            w1_sb = load_w(W["w1"], KD, DFF, "w1", offset=400)
            w2_sb = load_w(W["w2"], KF, D, "w2", offset=400)
            gT = act.tile([P, KF, TOK], bf16, tag="gT", bufs=1)
            xb2_t = []
            for t in range(NT):
                xb = xp.tile([P, D], f32, tag=f"xc{t}", bufs=1)
                nc.vector.tensor_add(xb, x_t[t], bod_sb[:, 1, :])
                xb2_t.append(xb)
            for m in range(KF):
                pa = ps.tile([P, TOK], f32, tag="mm")
                for k in range(KD):
                    nc.tensor.matmul(pa, w1_sb[:, k, m * P:(m + 1) * P],
                                     h2T[:, k, :],
                                     start=(k == 0), stop=(k == KD - 1))
                nc.scalar.activation(gT[:, m, :], pa, AF.Gelu,
                                     bias=b1_t[:, m:m + 1], scale=1.0)
            for t in range(NT):
                pz = ps.tile([P, D], f32, tag="mm")
                for k in range(KF):
                    nc.tensor.matmul(pz, gT[:, k, t * P:(t + 1) * P],
                                     w2_sb[:, k, :],
                                     start=(k == 0), stop=(k == KF - 1))
                xn = xp.tile([P, D], f32, tag=f"x{t}")
                nc.vector.tensor_add(xn, pz, xb2_t[t])
                x_t[t] = xn
                if debug:
                    nc.sync.dma_start(out=xdbg[l, t * P:(t + 1) * P, :], in_=xn)

        # ---- final LN + 8-way AllGather + lm_head ----
        hf = layernorm(x_t, "hf_")
        hfT = transpose_own(hf, "hfT")
        nc.sync.dma_start(
            out=bass.AP(tensor=af_i, offset=0,
                        ap=[[TOK, P], [P * TOK, KD], [1, TOK]]),
            in_=hfT)
        if sim:
            for rk in range(NC):
                nc.sync.dma_start(out=af_o[rk * D:(rk + 1) * D, :], in_=af_i[:, :])
        else:
            nc.gpsimd.collective_compute(
                "AllGather", Alu.bypass, replica_groups=GROUPS_ALL,
                ins=[af_i.ap().opt()], outs=[af_o.ap().opt()])

        xfT = act.tile([P, KD, NC * TOK], bf16, tag="xfT", bufs=1)
        for rk in range(NC):
            nc.sync.dma_start(
                out=xfT[:, :, rk * TOK:(rk + 1) * TOK],
                in_=bass.AP(tensor=af_o, offset=rk * D * TOK,
                            ap=[[TOK, P], [P * TOK, KD], [1, TOK]]))

        bh_t = sm.tile([P, vt], f32, tag="bh")
        nc.sync.dma_start(out=bh_t, in_=bass.AP(
            tensor=bhead, offset=0, ap=[[1, P], [P, vt]]))
        NCHUNK = (B * S) // 512
        MC = 13
        for m0 in range(0, vt, MC):
            mn = min(MC, vt - m0)
            woff = min(1500, max(tc.cur_priority - 8, 0))
            with tc.high_priority(offset=woff):
                whc = whp.tile([P, KD, MC * P], bf16, tag="wh", bufs=2)
                nc.sync.dma_start(
                    out=whc[:, :, :mn * P],
                    in_=bass.AP(tensor=whead, offset=m0 * P,
                                ap=[[vt * P, P], [P * vt * P, KD], [1, mn * P]]))
            for mi in range(mn):
                m = m0 + mi
                lo = act.tile([P, B * S], f32, tag=f"lo{m % 2}", bufs=1)
                for c2 in range(NCHUNK):
                    if (m + c2) % 2 == 0:
                        pl = ps.tile([P, 512], f32, tag="mm")
                    else:
                        pl = pss.tile([P, 512], f32, tag="sc", bufs=2)
                    for k in range(KD):
                        nc.tensor.matmul(
                            pl, whc[:, k, mi * P:(mi + 1) * P],
                            xfT[:, k, c2 * 512:(c2 + 1) * 512],
                            start=(k == 0), stop=(k == KD - 1))
                    dst = lo[:, c2 * 512:(c2 + 1) * 512]
                    if (m + c2) % 2 == 0:
                        nc.scalar.activation(dst, pl, AF.Identity,
                                             bias=bh_t[:, m:m + 1], scale=1.0)
                    else:
                        nc.vector.tensor_scalar_add(out=dst, in0=pl,
                                                    scalar1=bh_t[:, m:m + 1])
                nc.sync.dma_start(out=logits_out[m * P:(m + 1) * P, :], in_=lo)

    nc.compile()
    return nc


# --------------------------------------------------------------------------
# host side
# --------------------------------------------------------------------------

def host_prep(inputs, n_layers=L, vt=VT_FULL):
    emb = _f32(inputs["embedding"])
    pos = _f32(inputs["pos_embedding"])[0, :S]
    tokens = np.asarray(inputs["tokens"]).astype(np.int64)

    g1 = _f32(inputs["ln1_g"]); b1l = _f32(inputs["ln1_b"])
    g2 = _f32(inputs["ln2_g"]); b2l = _f32(inputs["ln2_b"])
    gf = _f32(inputs["lnf_g"]); bfl = _f32(inputs["lnf_b"])

    shared = {}
    for l in range(n_layers):
        Wq, Wk, Wv, Wo = (_f32(inputs[k][l]) for k in ["Wq", "Wk", "Wv", "Wo"])
        W1, W2 = _f32(inputs["W1"][l]), _f32(inputs["W2"][l])
        bq, bk, bv, bo = (_f32(inputs[k][l]) for k in ["bq", "bk", "bv", "bo"])
        b1, b2 = _f32(inputs["b1"][l]), _f32(inputs["b2"][l])
        shared[f"wq{l}"] = _bf(((Wq * g1[l]) / 8.0).T)
        shared[f"wk{l}"] = _bf((Wk * g1[l]).T)
        shared[f"wv{l}"] = _bf((Wv * g1[l]).T)
        shared[f"wo{l}"] = _bf(Wo.T)
        shared[f"w1{l}"] = _bf((W1 * g2[l]).T)
        shared[f"w2{l}"] = _bf(W2.T)
        bq_e = (bq + Wq @ b1l[l]) / 8.0
        bk_e = bk + Wk @ b1l[l]
        bv_e = bv + Wv @ b1l[l]
        b1_e = b1 + W1 @ b2l[l]
        bvec = np.zeros((P, 28), np.float32)
        bvec[:, 0:4] = bq_e.reshape(4, P).T
        bvec[:, 4:8] = bk_e.reshape(4, P).T
        bvec[:, 8:12] = bv_e.reshape(4, P).T
        bvec[:, 12:28] = b1_e.reshape(16, P).T
        shared[f"bvec{l}"] = bvec
        bod = np.zeros((2, P, D), np.float32)
        bod[0] = np.broadcast_to(bo, (P, D))
        bod[1] = np.broadcast_to(b2, (P, D))
        shared[f"bod{l}"] = bod

    Whead = _f32(inputs["Whead"]); bh = _f32(inputs["bhead"])
    Whead_eff = Whead * gf
    bh_eff = bh + Whead @ bfl

    in_maps = []
    for c in range(NC):
        b, j = c // 4, c % 4
        m = {"x0": np.zeros((TOK, D), np.float32)}
        for g in range(NT):
            t_ids = 512 * g + 4 * np.arange(P) + j
            m["x0"][g * P:(g + 1) * P] = emb[tokens[b, t_ids]] + pos[t_ids]
        mk = np.zeros((4, P, P), np.float32)
        for jp in range(4):
            rk = np.arange(P)[:, None]; rq = np.arange(P)[None, :]
            mk[jp] = (rk <= rq - (1 if jp > j else 0)).astype(np.float32)
        m["masks"] = _bf(mk)
        v0 = sum(VS[:c])
        n = min(VS[c], vt * P)
        wslice = np.zeros((D, vt * P), np.float32)
        bslice = np.zeros((vt * P,), np.float32)
        wslice[:, :n] = Whead_eff.T[:, v0:v0 + n]
        bslice[:n] = bh_eff[v0:v0 + n]
        m["whead"] = _bf(wslice)
        m["bhead"] = _f32(bslice)
        m.update(shared)
        in_maps.append(m)
    return in_maps


def assemble(results, vt=VT_FULL):
    gam = np.arange(NC * TOK)
    cp = gam // TOK; w = gam % TOK
    gp = w // P; rp = w % P
    bb = cp // 4; jj = cp % 4
    t = 512 * gp + 4 * rp + jj
    rows = bb * S + t
    out = np.empty((B * S, V), np.float32)
    for c in range(NC):
        v0 = sum(VS[:c])
        lt = results[c]["logitsT"][:VS[c]]
        out[rows, v0:v0 + VS[c]] = lt.T
    return out.reshape(B, S, V)


_CACHE = {}


def kernel(**inputs):
    key = ("full", L, VT_FULL)
    if key not in _CACHE:
        _CACHE[key] = build(L, VT_FULL, debug=False)
    nc = _CACHE[key]
    in_maps = host_prep(inputs, L, VT_FULL)
    res = run_bass_kernel_spmd(nc, in_maps, list(range(NC)))
    return assemble(res.results, VT_FULL)
